# revision 1
# baseline (speedup 1.0000x reference)
"""Trainium2 Bass kernel for nn_ExpressionEstimator_Attention.

Data-parallel across 8 NeuronCores: each core processes B/8 = 4096 samples.
Per-core pipeline (activations kept as (C_partitions, t, b) SBUF tiles):
  x load (rows on partitions) -> PE transposes to channel-major ->
  conv1..4 (2-t-windowed N=512 matmuls, taps accumulated in PSUM) ->
  DRAM-roundtrip reshape (the torch-faithful (B,64,9)->(B*9,64) row-major
  remix; both directions are large contiguous-run DMAs) -> fc1..3 ->
  att conv stack (taps K-stacked at 32-aligned partition groups via shifted
  SBUF replicas; one windowed matmul per 2 t) -> logits matmul (lw fused
  over t via K=9 stack) -> exp -> PE transposes to batch-partition layout ->
  softmax-normalized attention contraction -> prefetched indirect-DMA gather
  of mapping rows -> final (mapping x sub) einsum on DVE/GPSIMD.

Leaky ReLU (exact): r = Identity(0.02*psum + 0.02*b) on ACT, then
out = (psum + b) max r on DVE - one op per engine, bias via per-partition APs.

Matmul inputs default to float32r (TF32-like rounded fp32): 1 PE cycle/row
instead of 4 for full fp32. Measured end-to-end error vs the fp32 reference:
rel ~3e-4 (absmax ~1.5e-2 at |out|max ~46). Set KERNEL_F32R=0 for exact
fp32 matmuls (rel err ~5e-7, ~2.5x slower).
"""

import os
import sys

for _p in ("/opt/trn_rl_repo", "/root/.axon_site/_ro/trn_rl_repo"):
    if os.path.isdir(_p) and _p not in sys.path:
        sys.path.insert(0, _p)

import numpy as np

import concourse.bass as bass
import concourse.mybir as mybir
import concourse.tile as tile
from concourse import bacc
from concourse.masks import make_identity

FP = mybir.dt.float32
FR = mybir.dt.float32r
AX = mybir.AxisListType.X
OP = mybir.AluOpType
AF = mybir.ActivationFunctionType

NCORES = 8
B_FULL = 32768
T = 9
NA = 64
S = 32
NE = 53
NID = 5000

# (name, Cin, Cout) for the main conv stack
CONVS = [("c1", 64, 32), ("c2", 32, 64), ("c3", 64, 128), ("c4", 128, 64)]
FCS = [("f1", 64, 128, "leaky"), ("f2", 128, 64, "leaky"), ("f3", 64, 32, "tanh")]
ATTS = [("a1", 32, 16), ("a2", 16, 8), ("a3", 8, 4), ("a4", 4, 2), ("a5", 2, 1)]
WINDOWS = [(0, 2), (2, 2), (4, 2), (6, 2), (8, 1)]


def leaky_act(nc, rt, pm, out, bias2):
    """out = leaky(pm + b)  with bias2 = [[b, 0.02b]] per-partition.

    rt (SBUF scratch) <- ACT Identity(0.02*pm + 0.02*b)
    out <- DVE (pm + b) max rt
    """
    nc.scalar.activation(rt, pm, AF.Identity, scale=0.02, bias=bias2[:, 1:2])
    nc.vector.scalar_tensor_tensor(
        out=out, in0=pm, scalar=bias2[:, 0:1], in1=rt, op0=OP.add, op1=OP.max
    )


def emit(tc, io, B_core, NB, repeat=1, f32r=False):
    """Emit the per-core program. io: dict name -> DRAM AP."""
    nc = tc.nc
    MD = FR if f32r else FP  # matmul-input dtype (activations + weights)
    NCH = B_core // NB
    J = NB * T // 128
    assert NB * T % 128 == 0 and NB % 128 == 0
    NBU = NB // 128
    FLAT = NB * T

    import contextlib

    with contextlib.ExitStack() as ctx:
        const = ctx.enter_context(tc.tile_pool(name="const", bufs=1))
        xr_p = ctx.enter_context(tc.tile_pool(name="xr", bufs=2))
        st_p = ctx.enter_context(tc.tile_pool(name="stage", bufs=int(os.environ.get("BUF_STAGE", "6"))))
        rt_p = ctx.enter_context(tc.tile_pool(name="rt", bufs=int(os.environ.get("BUF_RT", "3"))))
        sm_p = ctx.enter_context(tc.tile_pool(name="small", bufs=3))
        m_p = ctx.enter_context(tc.tile_pool(name="gath", bufs=int(os.environ.get("BUF_MT", "4"))))
        ob_p = ctx.enter_context(tc.tile_pool(name="outb", bufs=2))
        dram = ctx.enter_context(tc.tile_pool(name="dram", bufs=2, space="DRAM"))
        rep_ps = {
            nm: ctx.enter_context(tc.tile_pool(name="repp_" + nm, bufs=1))
            for nm, _, _ in ATTS
        }
        ps_tr = ctx.enter_context(tc.tile_pool(name="ps_tr", bufs=2, space="PSUM"))
        ps_mm = ctx.enter_context(tc.tile_pool(name="ps_mm", bufs=int(os.environ.get("BUF_PM", "3")), space="PSUM"))
        ps_t2 = ctx.enter_context(tc.tile_pool(name="ps_t2", bufs=2, space="PSUM"))

        # ---------------- constants ----------------
        ident = const.tile([128, 128], FP)
        make_identity(nc, ident[:])

        wt = {}
        bt = {}

        def load_w(nm, shape):
            raw = const.tile(shape, FP, name="wraw_" + nm)
            nc.sync.dma_start(raw[:], io["w" + nm][:])
            if not f32r:
                return raw
            t = const.tile(shape, FR, name="w_" + nm)
            nc.vector.tensor_copy(t[:], raw[:])
            return t
        for nm, ci, co in CONVS:
            wt[nm] = load_w(nm, [ci, 3, co])
            bt[nm] = const.tile([co, 2], FP, name="b_" + nm)
            nc.sync.dma_start(bt[nm][:], io["b" + nm][:])
        for nm, ci, co, _ in FCS:
            wt[nm] = load_w(nm, [ci, co])
            bt[nm] = const.tile([co, 2], FP, name="b_" + nm)
            nc.sync.dma_start(bt[nm][:], io["b" + nm][:])
        for nm, ci, co in ATTS:
            wt[nm] = load_w(nm, [96, co])
            bt[nm] = const.tile([co, 2], FP, name="b_" + nm)
            nc.sync.dma_start(bt[nm][:], io["b" + nm][:])
        lw_raw = const.tile([T, T], FP)
        nc.sync.dma_start(lw_raw[:], io["lwT"][:])
        if f32r:
            lw_t = const.tile([T, T], FR)
            nc.vector.tensor_copy(lw_t[:], lw_raw[:])
        else:
            lw_t = lw_raw
        lb_t = const.tile([T, 1], FP)
        nc.sync.dma_start(lb_t[:], io["lb2"][:])

        idx_t = const.tile([128, NCH * NBU], mybir.dt.int32)
        nc.sync.dma_start(
            idx_t[:],
            io["idx"].rearrange("(col p) -> p col", p=128),
        )

        x_rows = io["x"].rearrange("b t c -> (b t) c")
        out_d = io["out"]
        map_d = io["map"]

        # ---------------- chunk loop ----------------
        for c in [cc for _r in range(repeat) for cc in range(NCH)]:
            # -- prefetch mapping gathers for this chunk (depends only on idx) --
            mts = []
            for u in range(NBU):
                mt = m_p.tile([128, NE * S], FP, tag="mt", name=f"mt_{c}_{u}")
                nc.gpsimd.indirect_dma_start(
                    out=mt[:],
                    out_offset=None,
                    in_=map_d[:],
                    in_offset=bass.IndirectOffsetOnAxis(
                        ap=idx_t[:, c * NBU + u : c * NBU + u + 1], axis=0
                    ),
                )
                mts.append(mt)

            # -- load x chunk, rows (b t) on partitions --
            xr = xr_p.tile([128, J, NA], FP)
            nc.sync.dma_start(
                xr[:],
                x_rows[c * FLAT : (c + 1) * FLAT, :].rearrange(
                    "(p j) c -> p j c", p=128
                ),
            )
            # -- transpose to X0 (64, t, b): adjacent (2g, 2g+1) -> (128,128) --
            x0 = st_p.tile([NA, T, NB], MD, tag="stage")
            for g in range(J // 2):
                ptr = ps_tr.tile([128, 128], FP)
                nc.tensor.transpose(
                    ptr[:],
                    xr[:, 2 * g : 2 * g + 2, :].rearrange("p q c -> p (q c)"),
                    ident[:128, :128],
                )
                for h in range(2):
                    j = 2 * g + h
                    t, bq = j % T, j // T
                    cp = nc.scalar.copy if h == 0 else nc.vector.tensor_copy
                    if h == 0:
                        nc.scalar.copy(out=x0[:, t, bq::NBU], in_=ptr[:NA, :])
                    else:
                        nc.vector.tensor_copy(x0[:, t, bq::NBU], ptr[NA:, :])

            # -- main convs: 2-t windowed matmuls (N=2*NB<=512) --
            cur = x0
            for nm, ci, co in CONVS:
                nxt = st_p.tile([co, T, NB], MD, tag="stage")
                for t0, L in WINDOWS:
                    pm = ps_mm.tile([co, 2, NB], FP, tag="pm")
                    full = [k for k in range(3) if 0 <= t0 + k - 1 and t0 + k - 1 + L <= T]
                    part = []
                    for k in range(3):
                        if k in full:
                            continue
                        # valid window positions for this boundary tap
                        lo = max(0, 1 - k - t0)
                        hi = min(L, T + 1 - k - t0)
                        if lo < hi:
                            part.append((k, lo, hi))
                    # full[0] opens the group, partials in the middle, the
                    # last full tap closes it (covers every element written).
                    nc.tensor.matmul(
                        pm[:, :L, :], wt[nm][:, full[0], :],
                        cur[:, t0 + full[0] - 1 : t0 + full[0] - 1 + L, :],
                        start=True, stop=False,
                    )
                    for k, lo, hi in part:
                        nc.tensor.matmul(
                            pm[:, lo:hi, :], wt[nm][:, k, :],
                            cur[:, t0 + lo + k - 1 : t0 + hi + k - 1, :],
                            start=False, stop=False,
                        )
                    for i, k in enumerate(full[1:]):
                        nc.tensor.matmul(
                            pm[:, :L, :], wt[nm][:, k, :],
                            cur[:, t0 + k - 1 : t0 + k - 1 + L, :],
                            start=False, stop=(i == len(full) - 2),
                        )
                    rt = rt_p.tile([128, 2 * NB], FP, tag="rt")
                    leaky_act(
                        nc, rt[:co, : L * NB],
                        pm[:, :L, :].rearrange("c l b -> c (l b)"),
                        nxt[:, t0 : t0 + L, :].rearrange("c l b -> c (l b)"),
                        bt[nm],
                    )
                cur = nxt

            # -- reshape via DRAM roundtrip: D2 row i = 9c+t = 64r+j --
            # split into column halves on the ACT HWDGE ring so each load
            # starts as soon as its half is stored, off the SP ring.
            d2 = dram.tile([NA * T, NB], MD)
            z = st_p.tile([64, T, NB], MD, tag="stage")
            H = NB // 2
            for h in range(2):
                cs = slice(h * H, (h + 1) * H)
                nc.sync.dma_start(
                    d2[:, cs].rearrange("(c t) b -> c t b", t=T),
                    cur[:, :, cs],
                )
                nc.sync.dma_start(
                    z[:, :, cs],
                    d2[:, cs].rearrange("(r j) b -> j r b", j=64),
                )

            # -- fc stack on flat (r b) columns --
            curf = z[:].rearrange("j r b -> j (r b)")
            for nm, ci, co, act in FCS:
                nxt = st_p.tile([co, FLAT], MD, tag="stage")
                for q0 in range(0, FLAT, 512):
                    q1 = min(q0 + 512, FLAT)
                    pmt = ps_mm.tile([co, 512], FP, tag="pm")
                    pm = pmt[:, : q1 - q0]
                    nc.tensor.matmul(pm, wt[nm][:], curf[:, q0:q1], start=True, stop=True)
                    if act == "leaky":
                        rt = rt_p.tile([128, 512], FP, tag="rt")
                        leaky_act(nc, rt[:co, : q1 - q0], pm, nxt[:, q0:q1], bt[nm])
                    else:
                        nc.scalar.activation(
                            nxt[:, q0:q1], pm, AF.Tanh, bias=bt[nm][:, 0:1]
                        )
                curf = nxt[:]

            rs = curf.rearrange("s (r b) -> s r b", b=NB)  # (32, 9, NB)

            # -- attention convs: shifted K-stacked replicas, 1 matmul per t --
            prev = rs
            for nm, ci, co in ATTS:
                rep = rep_ps[nm].tile([96, T, NB], MD, tag="rep")
                if c == 0:
                    # zero whole tile once (boundary slices + inter-k pad rows);
                    # this pool slot is exclusive to this layer so zeros persist.
                    nc.gpsimd.memset(rep[:].bitcast(FP), 0.0)
                nc.sync.dma_start(rep[0:ci, 1:T, :], prev[:, 0 : T - 1, :])
                nc.sync.dma_start(rep[32 : 32 + ci, :, :], prev[:, :, :])
                nc.sync.dma_start(rep[64 : 64 + ci, 0 : T - 1, :], prev[:, 1:T, :])
                nxt = st_p.tile([co, T, NB], MD, tag="stage")
                for t0, L in WINDOWS:
                    pm = ps_mm.tile([co, 2, NB], FP, tag="pm")
                    nc.tensor.matmul(
                        pm[:, :L, :], wt[nm][:], rep[:, t0 : t0 + L, :],
                        start=True, stop=True,
                    )
                    rt = rt_p.tile([128, 2 * NB], FP, tag="rt")
                    leaky_act(
                        nc, rt[:co, : L * NB],
                        pm[:, :L, :].rearrange("c l b -> c (l b)"),
                        nxt[:, t0 : t0 + L, :].rearrange("c l b -> c (l b)"),
                        bt[nm],
                    )
                prev = nxt[:]

            # -- logits: gather a into (9, NB) then one matmul --
            a5t = sm_p.tile([T, NB], MD, tag="a5t")
            nc.sync.dma_start(a5t[:], prev)
            pml = ps_mm.tile([T, NB], FP, tag="pm")
            nc.tensor.matmul(pml[:], lw_t[:], a5t[:], start=True, stop=True)
            et9 = sm_p.tile([T, NB], FP, tag="exp9")
            nc.scalar.activation(et9[:], pml[:], AF.Exp, bias=lb_t[:, 0:1])

            # -- per-128 subchunk: transpose to b-partitions, attention + output --
            for u in range(NBU):
                bsl = slice(u * 128, (u + 1) * 128)
                prt = ps_t2.tile([128, T, S], FP, tag="t2")
                for t in range(T):
                    nc.tensor.transpose(prt[:, t, :], rs[:, t, bsl].bitcast(FP), ident[:S, :S])
                pre = ps_t2.tile([128, T], FP, tag="t2")
                nc.tensor.transpose(pre[:], et9[:, bsl], ident[:T, :T])

                rsb = sm_p.tile([128, T, S], FP, tag="rsb")
                nc.vector.tensor_copy(rsb[:], prt[:])
                etb = sm_p.tile([128, T], FP, tag="etb")
                nc.scalar.copy(out=etb[:], in_=pre[:])

                den = sm_p.tile([128, 1], FP, tag="den")
                nc.vector.tensor_reduce(out=den[:], in_=etb[:], op=OP.add, axis=AX)
                rcp = sm_p.tile([128, 1], FP, tag="rcp")
                nc.vector.reciprocal(rcp[:], den[:])

                pp = sm_p.tile([128, S, T], FP, tag="pp")
                nc.gpsimd.tensor_tensor(
                    out=pp[:].rearrange("p s t -> p t s"),
                    in0=rsb[:],
                    in1=etb[:].unsqueeze(2).broadcast_to([128, T, S]),
                    op=OP.mult,
                )
                sub_u = sm_p.tile([128, S], FP, tag="subu")
                nc.vector.tensor_reduce(out=sub_u[:], in_=pp[:], op=OP.add, axis=AX)
                sub_n = sm_p.tile([128, S], FP, tag="subn")
                nc.vector.tensor_scalar(
                    out=sub_n[:], in0=sub_u[:], scalar1=rcp[:, 0:1], scalar2=10.0,
                    op0=OP.mult, op1=OP.mult,
                )

                mt = mts[u]
                pf = sm_p.tile([128, NE, S], FP, tag="pf")
                nc.gpsimd.tensor_tensor(
                    out=pf[:],
                    in0=mt[:].rearrange("p (e s) -> p e s", s=S),
                    in1=sub_n[:].unsqueeze(1).broadcast_to([128, NE, S]),
                    op=OP.mult,
                )
                ob = ob_p.tile([128, NE], FP, tag="ob")
                nc.vector.tensor_reduce(out=ob[:], in_=pf[:], op=OP.add, axis=AX)
                nc.sync.dma_start(out_d[c * NB + u * 128 : c * NB + (u + 1) * 128, :], ob[:])


def prep_host_inputs(inputs, B_core):
    """Transform weights to device layouts. Returns (common dict, per-core fn)."""
    com = {}
    for i, (nm, ci, co) in enumerate(CONVS, start=1):
        w = np.asarray(inputs[f"cw{i}"])
        b = np.asarray(inputs[f"cb{i}"])
        com["w" + nm] = np.ascontiguousarray(np.transpose(w, (1, 2, 0)))  # (Cin,3,Cout)
        com["b" + nm] = np.ascontiguousarray(np.stack([b, 0.02 * b], 1))
    for i, (nm, ci, co, _) in enumerate(FCS, start=1):
        w = np.asarray(inputs[f"fw{i}"])
        b = np.asarray(inputs[f"fb{i}"])
        com["w" + nm] = np.ascontiguousarray(w.T)  # (Cin, Cout)
        com["b" + nm] = np.ascontiguousarray(np.stack([b, 0.02 * b], 1))
    for i, (nm, ci, co) in enumerate(ATTS, start=1):
        w = np.asarray(inputs[f"aw{i}"])
        b = np.asarray(inputs[f"ab{i}"])
        wstk = np.zeros((96, co), np.float32)  # k-groups at 32-aligned rows
        for k in range(3):
            wstk[32 * k : 32 * k + ci, :] = w[:, :, k].T
        com["w" + nm] = wstk
        com["b" + nm] = np.ascontiguousarray(np.stack([b, 0.02 * b], 1))
    com["lwT"] = np.ascontiguousarray(np.asarray(inputs["lw"]).T)
    com["lb2"] = np.ascontiguousarray(np.asarray(inputs["lb"])[:, None])
    com["map"] = np.ascontiguousarray(
        np.asarray(inputs["mapping"])[0].reshape(NID, NE * S)
    )
    return com


_CACHE = {}


def _build(B_core, NB, num_devices, repeat=1, f32r=False):
    key = (B_core, NB, num_devices, repeat, f32r)
    if key in _CACHE:
        return _CACHE[key]
    nc = bacc.Bacc("TRN2", debug=False, num_devices=num_devices)
    io = {}
    io["x"] = nc.dram_tensor("x", [B_core, T, NA], FP, kind="ExternalInput").ap()
    io["idx"] = nc.dram_tensor("idx", [B_core], mybir.dt.int32, kind="ExternalInput").ap()
    io["map"] = nc.dram_tensor("map", [NID, NE * S], FP, kind="ExternalInput").ap()
    for nm, ci, co in CONVS:
        io["w" + nm] = nc.dram_tensor("w" + nm, [ci, 3, co], FP, kind="ExternalInput").ap()
        io["b" + nm] = nc.dram_tensor("b" + nm, [co, 2], FP, kind="ExternalInput").ap()
    for nm, ci, co, _ in FCS:
        io["w" + nm] = nc.dram_tensor("w" + nm, [ci, co], FP, kind="ExternalInput").ap()
        io["b" + nm] = nc.dram_tensor("b" + nm, [co, 2], FP, kind="ExternalInput").ap()
    for nm, ci, co in ATTS:
        io["w" + nm] = nc.dram_tensor("w" + nm, [96, co], FP, kind="ExternalInput").ap()
        io["b" + nm] = nc.dram_tensor("b" + nm, [co, 2], FP, kind="ExternalInput").ap()
    io["lwT"] = nc.dram_tensor("lwT", [T, T], FP, kind="ExternalInput").ap()
    io["lb2"] = nc.dram_tensor("lb2", [T, 1], FP, kind="ExternalInput").ap()
    io["out"] = nc.dram_tensor("out", [B_core, NE], FP, kind="ExternalOutput").ap()

    with tile.TileContext(nc) as tc:
        emit(tc, io, B_core, NB, repeat=repeat, f32r=f32r)
    nc.compile()
    _CACHE[key] = (nc, io)
    return nc, io


def kernel(**inputs):
    from concourse.bass_utils import run_bass_kernel_spmd

    x = np.ascontiguousarray(np.asarray(inputs["x"], dtype=np.float32))
    ids = np.ascontiguousarray(np.asarray(inputs["identity_id"], dtype=np.int32))
    B = x.shape[0]
    B_core = B // NCORES
    NB = 256
    com = prep_host_inputs(inputs, B_core)
    f32r = bool(int(os.environ.get("KERNEL_F32R", "1")))
    nc, _io = _build(B_core, NB, NCORES, f32r=f32r)
    in_maps = []
    for i in range(NCORES):
        m = dict(com)
        m["x"] = x[i * B_core : (i + 1) * B_core]
        m["idx"] = ids[i * B_core : (i + 1) * B_core]
        in_maps.append(m)
    trace = bool(int(os.environ.get("KERNEL_TRACE", "0")))
    kw = {}
    if trace:
        kw = dict(trace=True, tmpdir=os.environ.get("KERNEL_TRACE_DIR") or None)
    res = run_bass_kernel_spmd(nc, in_maps, list(range(NCORES)), **kw)
    global _LAST_RESULTS
    _LAST_RESULTS = res
    out = np.concatenate([res.results[i]["out"] for i in range(NCORES)], axis=0)
    return out.astype(np.float32)


_LAST_RESULTS = None

